# revision 20
# baseline (speedup 1.0000x reference)
"""Causal multi-head self-attention with RoPE on 8 Trainium2 NeuronCores.

Sharding: data-parallel over batch (B=2 -> 2 groups of 4 cores), tensor-
parallel over heads within each group (16 heads -> 4 heads/core). Each core
computes q/k/v projections for its 4 heads, RoPE, causal attention, and a
partial output projection; partials are summed across the 4 cores of a
batch group (host-side reduction in v1).

Math layout notes (per core):
  - everything is kept "transposed": xT [D, S], qT/kT [256, S], so matmuls
    contract over the partition dim with weights stationary.
  - scores are computed transposed, scT[kv, q], so softmax-exp feeds the
    AV matmul directly (no attention-matrix transpose). The softmax
    denominator comes from a ones-column appended to V. Max-subtraction is
    skipped: logits are bounded (|logit| < ~20) so exp is safe in fp32.
  - matmuls run in float32r (~1.6e-4 relerr, 4x faster than fp32 on PE).
"""

import numpy as np

B = 2
S = 2048
D = 1024
NH = 16
DK = 64
THETA = 10000.0
NCORES = 8
GROUP = 4          # cores per batch group (tensor-parallel over heads)
DQ = 256           # head dims per core (4 heads x 64)
NEG = -1.0e9

_CACHE = {}


def _host_tables(pos):
    pos = np.asarray(pos, dtype=np.float64)
    half = np.arange(0, DK, 2, dtype=np.float64) / DK          # (32,)
    inv_freq = 1.0 / (THETA ** half)                           # (32,)
    ang = pos[:, None] * inv_freq[None, :]                     # (S, 32)
    cos = np.cos(ang)
    sin = np.sin(ang)
    d = np.arange(128)
    ip = (d % DK) // 2                                         # pair index per row
    cosf = cos[:, ip].T.astype(np.float32).copy()              # (128, S)
    sinf = sin[:, ip].T.astype(np.float32).copy()
    pmat = np.zeros((128, 128), dtype=np.float32)
    for i in range(64):
        pmat[2 * i + 1, 2 * i] = -1.0                          # qswap[2i]   = -q[2i+1]
        pmat[2 * i, 2 * i + 1] = 1.0                           # qswap[2i+1] = +q[2i]
    ident = np.eye(128, dtype=np.float32)
    r = np.arange(128)
    trimask = (r[None, :] >= r[:, None]).astype(np.float32)
    return cosf, sinf, pmat, ident, trimask


def _vaug_pair_dst(vaug, m, j):
    return vaug[m][:, j, 0:130].rearrange(
        "p (hh n) -> p hh n", hh=2)[:, :, 0:64]

def _build_program():
    import concourse.bacc as bacc
    import concourse.mybir as mybir
    import concourse.tile as tile

    f32 = mybir.dt.float32
    f32r = mybir.dt.float32r
    EXP = mybir.ActivationFunctionType.Exp

    nc = bacc.Bacc("TRN2", target_bir_lowering=False)

    xT_d = nc.dram_tensor("xT", [D, S], f32, kind="ExternalInput")
    wqT_d = nc.dram_tensor("wqT", [D, DQ], f32, kind="ExternalInput")
    wkT_d = nc.dram_tensor("wkT", [D, DQ], f32, kind="ExternalInput")
    wvT_d = nc.dram_tensor("wvT", [D, DQ], f32, kind="ExternalInput")
    woT_d = nc.dram_tensor("woT", [DQ, D], f32, kind="ExternalInput")
    cosf_d = nc.dram_tensor("cosf", [128, S], f32, kind="ExternalInput")
    sinf_d = nc.dram_tensor("sinf", [128, S], f32, kind="ExternalInput")
    pmat_d = nc.dram_tensor("pmat", [128, 128], f32, kind="ExternalInput")
    ident_d = nc.dram_tensor("ident", [128, 128], f32, kind="ExternalInput")
    trim_d = nc.dram_tensor("trimask", [128, 128], f32, kind="ExternalInput")
    outp_d = nc.dram_tensor("outp", [D, S], f32, kind="ExternalOutput")

    KC = D // 128       # 8 contraction chunks
    MC = DQ // 128      # 2 head-dim chunks per core
    NJ = S // 128       # 16 kv chunks
    HALF = S // 2       # 1024

    with tile.TileContext(nc) as tc:
        with (
            tc.tile_pool(name="const", bufs=1) as constp,
            tc.tile_pool(name="wres", bufs=1) as wres,
            tc.tile_pool(name="big", bufs=1) as bigp,
        ):
            # ---- constants ----
            cosf = constp.tile([128, S], f32)
            sinf = constp.tile([128, S], f32)
            trim = constp.tile([128, 128], f32)
            ident = constp.tile([128, 128], f32)
            nc.gpsimd.dma_start(cosf[:], cosf_d[:])
            nc.gpsimd.dma_start(sinf[:], sinf_d[:])
            nc.gpsimd.dma_start(trim[:], trim_d[:])
            nc.gpsimd.dma_start(ident[:], ident_d[:])
            pstage = constp.tile([128, 128], f32)
            nc.gpsimd.dma_start(pstage[:], pmat_d[:])
            pmat = constp.tile([128, 128], f32r)
            nc.vector.tensor_copy(pmat[:], pstage[:])
            ones_f = constp.tile([128, 1], f32)
            nc.vector.memset(ones_f[:], 1.0)

            # ---- weights: load + round to f32r ----
            w_r = {}
            for name, dram in (("q", wqT_d), ("k", wkT_d), ("v", wvT_d)):
                st = constp.tile([128, KC * DQ], f32, tag="wstage")
                nc.sync.dma_start(
                    st[:].rearrange("p (k n) -> p k n", k=KC),
                    dram.rearrange("(k p) n -> p k n", p=128),
                )
                wr = wres.tile([128, KC, DQ], f32r, tag=f"w{name}")
                nc.vector.tensor_copy(wr[:], st[:].rearrange("p (k n) -> p k n", k=KC))
                w_r[name] = wr
            wost = constp.tile([128, MC * D], f32, tag="wstage")
            nc.sync.dma_start(
                wost[:].rearrange("p (c n) -> p c n", c=MC),
                woT_d.rearrange("(c p) n -> p c n", p=128),
            )
            wo_r = wres.tile([128, MC, 8, 128], f32r)
            nc.vector.tensor_copy(
                wo_r[:], wost[:].rearrange("p (c m n) -> p c m n", c=MC, m=8)
            )

            # ---- resident activations ----
            qT = bigp.tile([128, MC, S], f32r)     # becomes q_rot in place
            kT = bigp.tile([128, MC, S], f32r)     # becomes k_rot in place
            vT = bigp.tile([128, MC, S], f32)      # fp32: feeds PE transpose
            avT = bigp.tile([128, MC, S], f32r)    # attention output (pre o-proj)
            # [:, j, hh*65 + (0:64)] = v of head 2m+hh; col hh*65+64 = ones
            vaug = [bigp.tile([128, NJ, 130], f32r, tag=f"vaug{m}",
                              name=f"vaug{m}") for m in range(MC)]

            # ================= phase 1: q/k/v projections =================
            proj_targets = [
                ("q", qT), ("k", kT), ("v", vT),
            ]
            with (
                tc.tile_pool(name="xin", bufs=3) as xinp,
                tc.tile_pool(name="xr", bufs=3) as xrp,
                tc.tile_pool(name="psP", bufs=7, space="PSUM") as psP,
            ):
                for sq in range(4):                # S quarters of 512
                    s0 = sq * 512
                    accs = {}
                    for pname, _ in proj_targets:
                        for m in range(MC):
                            accs[(pname, m)] = psP.tile(
                                [128, 512], mybir.dt.float32, tag="acc",
                                name=f"acc_{pname}{m}")
                    for k in range(KC):
                        xs = xinp.tile([128, 512], f32)
                        nc.scalar.dma_start(xs[:], xT_d[k * 128:(k + 1) * 128,
                                                        s0:s0 + 512])
                        xr = xrp.tile([128, 512], f32r)
                        nc.gpsimd.tensor_copy(xr[:], xs[:])
                        for pname, _ in proj_targets:
                            for m in range(MC):
                                nc.tensor.matmul(
                                    accs[(pname, m)][:],
                                    w_r[pname][:, k, m * 128:(m + 1) * 128],
                                    xr[:],
                                    start=(k == 0), stop=(k == KC - 1),
                                )
                    for pname, dst in proj_targets:
                        for m in range(MC):
                            nc.vector.tensor_copy(dst[:, m, s0:s0 + 512],
                                                  accs[(pname, m)][:])

            # ============ phase 1.5: RoPE (in place) + v transpose ============
            with (
                tc.tile_pool(name="psSw", bufs=2, space="PSUM") as psSw,
                tc.tile_pool(name="psT", bufs=2, space="PSUM") as psT,
                tc.tile_pool(name="ropet", bufs=4) as ropet,
            ):
                for m in range(MC):
                    for src in (qT, kT):
                        for sh in range(2):
                            c0 = sh * HALF
                            sw = psSw.tile([128, HALF], mybir.dt.float32)
                            for s4 in range(2):
                                nc.tensor.matmul(
                                    sw[:, s4 * 512:(s4 + 1) * 512],
                                    pmat[:],
                                    src[:, m, c0 + s4 * 512:c0 + (s4 + 1) * 512],
                                )
                            t1 = ropet.tile([128, HALF], f32, tag="t1")
                            nc.gpsimd.tensor_mul(
                                t1[:], src[:, m, c0:c0 + HALF].bitcast(f32),
                                cosf[:, c0:c0 + HALF])
                            t2 = ropet.tile([128, HALF], f32, tag="t2")
                            nc.vector.tensor_mul(t2[:], sw[:], sinf[:, c0:c0 + HALF])
                            nc.vector.tensor_add(src[:, m, c0:c0 + HALF], t1[:], t2[:])
                # v transpose: vT [2head-dims, kv] -> v natural chunks in vaug
                for m in range(MC):
                    for j in range(NJ):
                        pt = psT.tile([128, 128], f32)
                        nc.tensor.transpose(pt[:], vT[:, m, j * 128:(j + 1) * 128],
                                            ident[:])
                        nc.scalar.copy(
                            _vaug_pair_dst(vaug, m, j),
                            pt[:].rearrange("p (hh n) -> p hh n", hh=2))
                for m in range(MC):
                    nc.gpsimd.tensor_copy(
                        vaug[m][:, :, 64:130:65],
                        ones_f[:, None, :].broadcast_to([128, NJ, 2]),
                    )

            # ================= phase 2: attention =================
            with (
                tc.tile_pool(name="av", bufs=2, space="PSUM") as avp,
                tc.tile_pool(name="seg", bufs=3, space="PSUM") as segp,
                tc.tile_pool(name="attn", bufs=3) as attnp,
                tc.tile_pool(name="smax", bufs=4) as smaxp,
            ):
                def avmm(av, h, j, kv0, at, q0, jmax):
                    for sg in range(2):
                        sb0 = q0 + sg * 512
                        sb1 = sb0 + 512
                        lo = max(sb0, kv0)
                        if lo >= sb1:
                            continue
                        nc.tensor.matmul(
                            av[:, lo - q0:sb1 - q0],
                            vaug[h // 2][:, j, (h % 2) * 65:(h % 2) * 65 + 65],
                            at[:, lo - q0:sb1 - q0],
                            start=(j == 0),
                            stop=(j == min(jmax, sb1 // 128) - 1),
                        )

                for m in range(MC):
                    for hh in range(2):
                        h = 2 * m + hh
                        p0 = hh * DK
                        for H in range(2):
                            q0 = H * HALF
                            jmax = (H + 1) * (NJ // 2)
                            av = avp.tile([DK + 1, HALF], mybir.dt.float32)
                            pend = []
                            for j in range(jmax):
                                kv0 = j * 128
                                at = attnp.tile([128, HALF], f32r)
                                for sg in range(2):
                                    sb0 = q0 + sg * 512       # seg bounds (abs cols)
                                    sb1 = sb0 + 512
                                    lo = max(sb0, kv0)
                                    if lo >= sb1:
                                        continue
                                    slen = sb1 - lo
                                    sc = segp.tile([128, 512], mybir.dt.float32)
                                    nc.tensor.matmul(
                                        sc[:, 0:slen],
                                        kT[p0:p0 + DK, m, kv0:kv0 + 128],
                                        qT[p0:p0 + DK, m, lo:lo + slen],
                                    )
                                    nc.scalar.activation(
                                        at[:, lo - q0:lo - q0 + slen],
                                        sc[:, 0:slen], EXP, scale=0.125)
                                    if lo == kv0:             # diagonal block
                                        dg = at[:, lo - q0:lo - q0 + 128]
                                        nc.gpsimd.tensor_mul(
                                            dg, dg.bitcast(f32), trim[:])
                                pend.append((h, j, kv0, at))
                                while pend:
                                    avmm(av, *pend.pop(0), q0, jmax)
                            rec = smaxp.tile([1, HALF], f32, tag="rec")
                            nc.vector.reciprocal(rec[:], av[DK:DK + 1, :])
                            bc = smaxp.tile([DK, HALF], f32, tag="bc")
                            nc.gpsimd.partition_broadcast(bc[:], rec[:])
                            nc.vector.tensor_mul(
                                avT[p0:p0 + DK, m, q0:q0 + HALF],
                                av[0:DK, :], bc[:])

            # ================= phase 3: output projection =================
            with (
                tc.tile_pool(name="psO", bufs=2, space="PSUM") as psO,
                tc.tile_pool(name="ost", bufs=3) as ostp,
            ):
                for mo in range(8):
                    for sh in range(2):
                        c0 = sh * HALF
                        po = psO.tile([128, HALF], mybir.dt.float32)
                        for c in range(MC):
                            for s4 in range(2):
                                nc.tensor.matmul(
                                    po[:, s4 * 512:(s4 + 1) * 512],
                                    wo_r[:, c, mo, :],
                                    avT[:, c, c0 + s4 * 512:c0 + (s4 + 1) * 512],
                                    start=(c == 0), stop=(c == MC - 1),
                                )
                        ot = ostp.tile([128, HALF], f32)
                        nc.vector.tensor_copy(ot[:], po[:])
                        nc.sync.dma_start(
                            outp_d[mo * 128:(mo + 1) * 128, c0:c0 + HALF], ot[:])

    nc.compile()
    return nc


def get_program():
    if "nc" not in _CACHE:
        _CACHE["nc"] = _build_program()
    return _CACHE["nc"]


def make_in_maps(x, wq, wk, wv, wo, token_positions):
    cosf, sinf, pmat, ident, trimask = _host_tables(token_positions)
    x = np.asarray(x, dtype=np.float32)
    wq = np.asarray(wq, dtype=np.float32)
    wk = np.asarray(wk, dtype=np.float32)
    wv = np.asarray(wv, dtype=np.float32)
    wo = np.asarray(wo, dtype=np.float32)
    in_maps = []
    for g in range(NCORES):
        b = g // GROUP
        hg = g % GROUP
        sl = slice(hg * DQ, (hg + 1) * DQ)
        in_maps.append({
            "xT": np.ascontiguousarray(x[b].T),
            "wqT": np.ascontiguousarray(wq[sl, :].T),
            "wkT": np.ascontiguousarray(wk[sl, :].T),
            "wvT": np.ascontiguousarray(wv[sl, :].T),
            "woT": np.ascontiguousarray(wo[:, sl].T),
            "cosf": cosf, "sinf": sinf, "pmat": pmat,
            "ident": ident, "trimask": trimask,
        })
    return in_maps


def kernel(x, token_positions, wq, wk, wv, wo):
    from concourse.bass_utils import run_bass_kernel_spmd

    # token_positions is always arange(S) for this problem size; the rope
    # tables are built from it on the host either way.
    tp = np.asarray(token_positions)
    assert tp.shape == (S,)

    nc = get_program()
    in_maps = make_in_maps(x, wq, wk, wv, wo, tp)
    res = run_bass_kernel_spmd(nc, in_maps, list(range(NCORES)))
    out = np.zeros((B, S, D), dtype=np.float32)
    for b in range(B):
        acc = np.zeros((D, S), dtype=np.float32)
        for g in range(b * GROUP, (b + 1) * GROUP):
            acc += res.results[g]["outp"]
        out[b] = acc.T
    return out


# revision 28
# speedup vs baseline: 1.0193x; 1.0193x over previous
"""Causal multi-head self-attention with RoPE on 8 Trainium2 NeuronCores.

Sharding: data-parallel over batch (B=2 -> 2 groups of 4 cores), tensor-
parallel over heads within each group (16 heads -> 4 heads/core). Each core
computes q/k/v projections for its 4 heads, RoPE, causal attention, and a
partial output projection; partials are summed across the 4 cores of a
batch group (host-side reduction in v1).

Math layout notes (per core):
  - everything is kept "transposed": xT [D, S], qT/kT [256, S], so matmuls
    contract over the partition dim with weights stationary.
  - scores are computed transposed, scT[kv, q], so softmax-exp feeds the
    AV matmul directly (no attention-matrix transpose). The softmax
    denominator comes from a ones-column appended to V. Max-subtraction is
    skipped: logits are bounded (|logit| < ~20) so exp is safe in fp32.
  - matmuls run in float32r (~1.6e-4 relerr, 4x faster than fp32 on PE).
"""

import numpy as np

B = 2
S = 2048
D = 1024
NH = 16
DK = 64
THETA = 10000.0
NCORES = 8
GROUP = 4          # cores per batch group (tensor-parallel over heads)
DQ = 256           # head dims per core (4 heads x 64)
NEG = -1.0e9

_CACHE = {}


def _host_tables(pos):
    pos = np.asarray(pos, dtype=np.float64)
    half = np.arange(0, DK, 2, dtype=np.float64) / DK          # (32,)
    inv_freq = 1.0 / (THETA ** half)                           # (32,)
    ang = pos[:, None] * inv_freq[None, :]                     # (S, 32)
    cos = np.cos(ang)
    sin = np.sin(ang)
    d = np.arange(128)
    ip = (d % DK) // 2                                         # pair index per row
    cosf = cos[:, ip].T.astype(np.float32).copy()              # (128, S)
    sinf = sin[:, ip].T.astype(np.float32).copy()
    pmat = np.zeros((128, 128), dtype=np.float32)
    for i in range(64):
        pmat[2 * i + 1, 2 * i] = -1.0                          # qswap[2i]   = -q[2i+1]
        pmat[2 * i, 2 * i + 1] = 1.0                           # qswap[2i+1] = +q[2i]
    ident = np.eye(128, dtype=np.float32)
    r = np.arange(128)
    trimask = (r[None, :] >= r[:, None]).astype(np.float32)
    return cosf, sinf, pmat, ident, trimask


def _vaug_pair_dst(vaug, m, j):
    return vaug[m][:, j, 0:130].rearrange(
        "p (hh n) -> p hh n", hh=2)[:, :, 0:64]

def _build_program():
    import concourse.bacc as bacc
    import concourse.mybir as mybir
    import concourse.tile as tile

    f32 = mybir.dt.float32
    f32r = mybir.dt.float32r
    EXP = mybir.ActivationFunctionType.Exp

    nc = bacc.Bacc("TRN2", target_bir_lowering=False)

    xT_d = nc.dram_tensor("xT", [D, S], f32, kind="ExternalInput")
    wqT_d = nc.dram_tensor("wqT", [D, DQ], f32, kind="ExternalInput")
    wkT_d = nc.dram_tensor("wkT", [D, DQ], f32, kind="ExternalInput")
    wvT_d = nc.dram_tensor("wvT", [D, DQ], f32, kind="ExternalInput")
    woT_d = nc.dram_tensor("woT", [DQ, D], f32, kind="ExternalInput")
    cosf_d = nc.dram_tensor("cosf", [128, S], f32, kind="ExternalInput")
    sinf_d = nc.dram_tensor("sinf", [128, S], f32, kind="ExternalInput")
    pmat_d = nc.dram_tensor("pmat", [128, 128], f32, kind="ExternalInput")
    ident_d = nc.dram_tensor("ident", [128, 128], f32, kind="ExternalInput")
    trim_d = nc.dram_tensor("trimask", [128, 128], f32, kind="ExternalInput")
    outp_d = nc.dram_tensor("outp", [D, S], f32, kind="ExternalOutput")

    KC = D // 128       # 8 contraction chunks
    MC = DQ // 128      # 2 head-dim chunks per core
    NJ = S // 128       # 16 kv chunks
    HALF = S // 2       # 1024

    with tile.TileContext(nc) as tc:
        with (
            tc.tile_pool(name="const", bufs=1) as constp,
            tc.tile_pool(name="wres", bufs=1) as wres,
            tc.tile_pool(name="big", bufs=1) as bigp,
        ):
            # ---- constants ----
            cosf = constp.tile([128, S], f32)
            sinf = constp.tile([128, S], f32)
            trim = constp.tile([128, 128], f32)
            ident = constp.tile([128, 128], f32)
            pstage = constp.tile([128, 128], f32)
            pmat = constp.tile([128, 128], f32r)
            ones_f = constp.tile([128, 1], f32)
            nc.vector.memset(ones_f[:], 1.0)

            # ---- weights: chunked load + round so k=0 matmuls start early
            w_r = {}
            wst = {}
            for name, dram in (("q", wqT_d), ("k", wkT_d), ("v", wvT_d)):
                wst[name] = constp.tile([128, KC * DQ], f32, tag=f"wst{name}",
                                        name=f"wst_{name}")
                w_r[name] = wres.tile([128, KC, DQ], f32r, tag=f"w{name}",
                                      name=f"w_{name}")
            for k in range(KC):
                for name, dram in (("q", wqT_d), ("k", wkT_d), ("v", wvT_d)):
                    st = wst[name][:].rearrange("p (k n) -> p k n", k=KC)
                    nc.sync.dma_start(st[:, k, :],
                                      dram[k * 128:(k + 1) * 128, :])
                    nc.vector.tensor_copy(w_r[name][:, k, :], st[:, k, :])

            # ---- resident activations ----
            qT = bigp.tile([128, MC, S], f32r)     # becomes q_rot in place
            kT = bigp.tile([128, MC, S], f32r)     # becomes k_rot in place
            vT = bigp.tile([128, MC, S], f32)      # fp32: feeds PE transpose
            avT = bigp.tile([128, MC, S], f32r)    # attention output (pre o-proj)
            # [:, j, hh*65 + (0:64)] = v of head 2m+hh; col hh*65+64 = ones
            vaug = [bigp.tile([128, NJ, 130], f32r, tag=f"vaug{m}",
                              name=f"vaug{m}") for m in range(MC)]

            # ================= phase 1: q/k/v projections =================
            proj_targets = [
                ("q", qT), ("k", kT), ("v", vT),
            ]
            with (
                tc.tile_pool(name="xin", bufs=5) as xinp,
                tc.tile_pool(name="xr", bufs=5) as xrp,
                tc.tile_pool(name="psP", bufs=8, space="PSUM") as psP,
            ):
                for sq in range(4):                # S quarters of 512
                    s0 = sq * 512
                    accs = {}
                    for pname, _ in proj_targets:
                        for m in range(MC):
                            accs[(pname, m)] = psP.tile(
                                [128, 512], mybir.dt.float32, tag="acc",
                                name=f"acc_{pname}{m}")
                    for k in range(KC):
                        xs = xinp.tile([128, 512], f32)
                        dmae = nc.scalar if k % 2 == 0 else nc.gpsimd
                        dmae.dma_start(xs[:], xT_d[k * 128:(k + 1) * 128,
                                                   s0:s0 + 512])
                        xr = xrp.tile([128, 512], f32r)
                        nc.gpsimd.tensor_copy(xr[:], xs[:])
                        for pname, _ in proj_targets:
                            for m in range(MC):
                                nc.tensor.matmul(
                                    accs[(pname, m)][:],
                                    w_r[pname][:, k, m * 128:(m + 1) * 128],
                                    xr[:],
                                    start=(k == 0), stop=(k == KC - 1),
                                )
                    for pname, dst in proj_targets:
                        for m in range(MC):
                            nc.vector.tensor_copy(dst[:, m, s0:s0 + 512],
                                                  accs[(pname, m)][:])

            # tables arrive on the SWDGE queue behind the odd x chunks
            nc.gpsimd.dma_start(cosf[:], cosf_d[:])
            nc.gpsimd.dma_start(sinf[:], sinf_d[:])
            nc.gpsimd.dma_start(trim[:], trim_d[:])
            nc.gpsimd.dma_start(ident[:], ident_d[:])
            nc.gpsimd.dma_start(pstage[:], pmat_d[:])
            nc.vector.tensor_copy(pmat[:], pstage[:])

            # ============ phase 1.5: RoPE (in place) + v transpose ============
            with (
                tc.tile_pool(name="psSw", bufs=2, space="PSUM") as psSw,
                tc.tile_pool(name="psT", bufs=2, space="PSUM") as psT,
                tc.tile_pool(name="ropet", bufs=4) as ropet,
            ):
                for m in range(MC):
                    for src in (qT, kT):
                        for sh in range(2):
                            c0 = sh * HALF
                            sw = psSw.tile([128, HALF], mybir.dt.float32)
                            for s4 in range(2):
                                nc.tensor.matmul(
                                    sw[:, s4 * 512:(s4 + 1) * 512],
                                    pmat[:],
                                    src[:, m, c0 + s4 * 512:c0 + (s4 + 1) * 512],
                                )
                            t1 = ropet.tile([128, HALF], f32, tag="t1")
                            nc.gpsimd.tensor_mul(
                                t1[:], src[:, m, c0:c0 + HALF].bitcast(f32),
                                cosf[:, c0:c0 + HALF])
                            t2 = ropet.tile([128, HALF], f32, tag="t2")
                            nc.vector.tensor_mul(t2[:], sw[:], sinf[:, c0:c0 + HALF])
                            nc.vector.tensor_add(src[:, m, c0:c0 + HALF], t1[:], t2[:])
                # v transpose: vT [2head-dims, kv] -> v natural chunks in vaug
                for m in range(MC):
                    nc.gpsimd.tensor_copy(
                        vaug[m][:, :, 64:130:65],
                        ones_f[:, None, :].broadcast_to([128, NJ, 2]),
                    )
                for m in range(MC):
                    for j in range(NJ):
                        pt = psT.tile([128, 128], f32)
                        nc.tensor.transpose(pt[:], vT[:, m, j * 128:(j + 1) * 128],
                                            ident[:])
                        nc.scalar.copy(
                            _vaug_pair_dst(vaug, m, j),
                            pt[:].rearrange("p (hh n) -> p hh n", hh=2))

            # ---- wo load (sync queue, behind the qkv chunks) + round ----
            wost = constp.tile([128, MC * D], f32, tag="wstage")
            nc.sync.dma_start(
                wost[:].rearrange("p (c n) -> p c n", c=MC),
                woT_d.rearrange("(c p) n -> p c n", p=128),
            )
            wo_r = wres.tile([128, MC, 8, 128], f32r)
            nc.vector.tensor_copy(
                wo_r[:], wost[:].rearrange("p (c m n) -> p c m n", c=MC, m=8)
            )

            # ================= phase 2: attention =================
            with (
                tc.tile_pool(name="av", bufs=2, space="PSUM") as avp,
                tc.tile_pool(name="seg", bufs=2, space="PSUM") as segp,
                tc.tile_pool(name="attn", bufs=3) as attnp,
                tc.tile_pool(name="smax", bufs=4) as smaxp,
            ):
                def avmm(av, h, j, kv0, at, q0, jmax):
                    for sg in range(2):
                        sb0 = q0 + sg * 512
                        sb1 = sb0 + 512
                        lo = max(sb0, kv0)
                        if lo >= sb1:
                            continue
                        nc.tensor.matmul(
                            av[:, lo - q0:sb1 - q0],
                            vaug[h // 2][:, j, (h % 2) * 65:(h % 2) * 65 + 65],
                            at[:, lo - q0:sb1 - q0],
                            start=(j == 0),
                            stop=(j == min(jmax, sb1 // 128) - 1),
                        )

                for m in range(MC):
                    for hh in range(2):
                        h = 2 * m + hh
                        p0 = hh * DK
                        for H in range(2):
                            q0 = H * HALF
                            jmax = (H + 1) * (NJ // 2)
                            av = avp.tile([DK + 1, HALF], mybir.dt.float32)
                            pend = []
                            for j in range(jmax):
                                kv0 = j * 128
                                at = attnp.tile([128, HALF], f32r)
                                jlo = max(q0, kv0)            # valid q start
                                sc = segp.tile([128, HALF], mybir.dt.float32)
                                for sg in range(2):
                                    sb0 = q0 + sg * 512       # seg bounds (abs cols)
                                    sb1 = sb0 + 512
                                    lo = max(sb0, kv0)
                                    if lo >= sb1:
                                        continue
                                    nc.tensor.matmul(
                                        sc[:, lo - q0:sb1 - q0],
                                        kT[p0:p0 + DK, m, kv0:kv0 + 128],
                                        qT[p0:p0 + DK, m, lo:lo + sb1 - lo],
                                    )
                                nc.scalar.activation(
                                    at[:, jlo - q0:HALF],
                                    sc[:, jlo - q0:HALF], EXP, scale=0.125)
                                if jlo == kv0:                # diagonal block
                                    dg = at[:, jlo - q0:jlo - q0 + 128]
                                    nc.gpsimd.tensor_mul(
                                        dg, dg.bitcast(f32), trim[:])
                                pend.append((h, j, kv0, at))
                                while pend:
                                    avmm(av, *pend.pop(0), q0, jmax)
                            rec = smaxp.tile([1, HALF], f32, tag="rec")
                            nc.vector.reciprocal(rec[:], av[DK:DK + 1, :])
                            bc = smaxp.tile([DK, HALF], f32, tag="bc")
                            nc.gpsimd.partition_broadcast(bc[:], rec[:])
                            nc.vector.tensor_mul(
                                avT[p0:p0 + DK, m, q0:q0 + HALF],
                                av[0:DK, :], bc[:])

            # ================= phase 3: output projection =================
            with (
                tc.tile_pool(name="psO", bufs=2, space="PSUM") as psO,
                tc.tile_pool(name="ost", bufs=3) as ostp,
            ):
                for mo in range(8):
                    for sh in range(2):
                        c0 = sh * HALF
                        po = psO.tile([128, HALF], mybir.dt.float32)
                        for c in range(MC):
                            for s4 in range(2):
                                nc.tensor.matmul(
                                    po[:, s4 * 512:(s4 + 1) * 512],
                                    wo_r[:, c, mo, :],
                                    avT[:, c, c0 + s4 * 512:c0 + (s4 + 1) * 512],
                                    start=(c == 0), stop=(c == MC - 1),
                                )
                        ot = ostp.tile([128, HALF], f32)
                        if (2 * mo + sh) % 2 == 0:
                            nc.scalar.copy(ot[:], po[:])
                        else:
                            nc.vector.tensor_copy(ot[:], po[:])
                        nc.sync.dma_start(
                            outp_d[mo * 128:(mo + 1) * 128, c0:c0 + HALF], ot[:])

    nc.compile()
    return nc


def get_program():
    if "nc" not in _CACHE:
        _CACHE["nc"] = _build_program()
    return _CACHE["nc"]


def make_in_maps(x, wq, wk, wv, wo, token_positions):
    cosf, sinf, pmat, ident, trimask = _host_tables(token_positions)
    x = np.asarray(x, dtype=np.float32)
    wq = np.asarray(wq, dtype=np.float32)
    wk = np.asarray(wk, dtype=np.float32)
    wv = np.asarray(wv, dtype=np.float32)
    wo = np.asarray(wo, dtype=np.float32)
    in_maps = []
    for g in range(NCORES):
        b = g // GROUP
        hg = g % GROUP
        sl = slice(hg * DQ, (hg + 1) * DQ)
        in_maps.append({
            "xT": np.ascontiguousarray(x[b].T),
            "wqT": np.ascontiguousarray(wq[sl, :].T),
            "wkT": np.ascontiguousarray(wk[sl, :].T),
            "wvT": np.ascontiguousarray(wv[sl, :].T),
            "woT": np.ascontiguousarray(wo[:, sl].T),
            "cosf": cosf, "sinf": sinf, "pmat": pmat,
            "ident": ident, "trimask": trimask,
        })
    return in_maps


def kernel(x, token_positions, wq, wk, wv, wo):
    from concourse.bass_utils import run_bass_kernel_spmd

    # token_positions is always arange(S) for this problem size; the rope
    # tables are built from it on the host either way.
    tp = np.asarray(token_positions)
    assert tp.shape == (S,)

    nc = get_program()
    in_maps = make_in_maps(x, wq, wk, wv, wo, tp)
    res = run_bass_kernel_spmd(nc, in_maps, list(range(NCORES)))
    out = np.zeros((B, S, D), dtype=np.float32)
    for b in range(B):
        acc = np.zeros((D, S), dtype=np.float32)
        for g in range(b * GROUP, (b + 1) * GROUP):
            acc += res.results[g]["outp"]
        out[b] = acc.T
    return out


# revision 34
# speedup vs baseline: 1.0394x; 1.0197x over previous
"""Causal multi-head self-attention with RoPE on 8 Trainium2 NeuronCores.

Sharding: data-parallel over batch (B=2 -> 2 groups of 4 cores), tensor-
parallel over heads within each group (16 heads -> 4 heads/core). Each core
computes q/k/v projections for its 4 heads, RoPE, causal attention, and a
partial output projection; partials are summed across the 4 cores of a
batch group (host-side reduction in v1).

Math layout notes (per core):
  - everything is kept "transposed": xT [D, S], qT/kT [256, S], so matmuls
    contract over the partition dim with weights stationary.
  - scores are computed transposed, scT[kv, q], so softmax-exp feeds the
    AV matmul directly (no attention-matrix transpose). The softmax
    denominator comes from a ones-column appended to V. Max-subtraction is
    skipped: logits are bounded (|logit| < ~20) so exp is safe in fp32.
  - matmuls run in float32r (~1.6e-4 relerr, 4x faster than fp32 on PE).
"""

import numpy as np

B = 2
S = 2048
D = 1024
NH = 16
DK = 64
THETA = 10000.0
NCORES = 8
GROUP = 4          # cores per batch group (tensor-parallel over heads)
DQ = 256           # head dims per core (4 heads x 64)
NEG = -1.0e9

_CACHE = {}


def _host_tables(pos):
    pos = np.asarray(pos, dtype=np.float64)
    half = np.arange(0, DK, 2, dtype=np.float64) / DK          # (32,)
    inv_freq = 1.0 / (THETA ** half)                           # (32,)
    ang = pos[:, None] * inv_freq[None, :]                     # (S, 32)
    cos = np.cos(ang)
    sin = np.sin(ang)
    d = np.arange(128)
    ip = (d % DK) // 2                                         # pair index per row
    cosf = cos[:, ip].T.astype(np.float32).copy()              # (128, S)
    sinf = sin[:, ip].T.astype(np.float32).copy()
    pmat = np.zeros((128, 128), dtype=np.float32)
    for i in range(64):
        pmat[2 * i + 1, 2 * i] = -1.0                          # qswap[2i]   = -q[2i+1]
        pmat[2 * i, 2 * i + 1] = 1.0                           # qswap[2i+1] = +q[2i]
    ident = np.eye(128, dtype=np.float32)
    r = np.arange(128)
    trimask = (r[None, :] >= r[:, None]).astype(np.float32)
    return cosf, sinf, pmat, ident, trimask


def _vaug_pair_dst(vaug, m, j):
    return vaug[m][:, j, 0:130].rearrange(
        "p (hh n) -> p hh n", hh=2)[:, :, 0:64]

def _build_program():
    import concourse.bacc as bacc
    import concourse.mybir as mybir
    import concourse.tile as tile

    f32 = mybir.dt.float32
    f32r = mybir.dt.float32r
    EXP = mybir.ActivationFunctionType.Exp

    nc = bacc.Bacc("TRN2", target_bir_lowering=False)

    xT_d = nc.dram_tensor("xT", [D, S], f32, kind="ExternalInput")
    wqT_d = nc.dram_tensor("wqT", [D, DQ], f32, kind="ExternalInput")
    wkT_d = nc.dram_tensor("wkT", [D, DQ], f32, kind="ExternalInput")
    wvT_d = nc.dram_tensor("wvT", [D, DQ], f32, kind="ExternalInput")
    woT_d = nc.dram_tensor("woT", [DQ, D], f32, kind="ExternalInput")
    cosf_d = nc.dram_tensor("cosf", [128, S], f32, kind="ExternalInput")
    sinf_d = nc.dram_tensor("sinf", [128, S], f32, kind="ExternalInput")
    pmat_d = nc.dram_tensor("pmat", [128, 128], f32, kind="ExternalInput")
    ident_d = nc.dram_tensor("ident", [128, 128], f32, kind="ExternalInput")
    trim_d = nc.dram_tensor("trimask", [128, 128], f32, kind="ExternalInput")
    outp_d = nc.dram_tensor("outp", [D, S], f32, kind="ExternalOutput")

    KC = D // 128       # 8 contraction chunks
    MC = DQ // 128      # 2 head-dim chunks per core
    NJ = S // 128       # 16 kv chunks
    HALF = S // 2       # 1024

    with tile.TileContext(nc) as tc:
        with (
            tc.tile_pool(name="const", bufs=1) as constp,
            tc.tile_pool(name="wres", bufs=1) as wres,
            tc.tile_pool(name="big", bufs=1) as bigp,
        ):
            # ---- constants ----
            cosf = constp.tile([128, S], f32)
            sinf = constp.tile([128, S], f32)
            trim = constp.tile([128, 128], f32)
            ident = constp.tile([128, 128], f32)
            pstage = constp.tile([128, 128], f32)
            pmat = constp.tile([128, 128], f32r)
            ones_f = constp.tile([128, 1], f32)
            nc.vector.memset(ones_f[:], 1.0)

            # ---- weights: chunked load + round so k=0 matmuls start early
            w_r = {}
            wst = {}
            for name, dram in (("q", wqT_d), ("k", wkT_d), ("v", wvT_d)):
                wst[name] = constp.tile([128, KC * DQ], f32, tag=f"wst{name}",
                                        name=f"wst_{name}")
                w_r[name] = wres.tile([128, KC, DQ], f32r, tag=f"w{name}",
                                      name=f"w_{name}")
            for k in range(KC):
                for name, dram in (("q", wqT_d), ("k", wkT_d), ("v", wvT_d)):
                    st = wst[name][:].rearrange("p (k n) -> p k n", k=KC)
                    nc.sync.dma_start(st[:, k, :],
                                      dram[k * 128:(k + 1) * 128, :])
                    nc.vector.tensor_copy(w_r[name][:, k, :], st[:, k, :])

            # ---- resident activations ----
            qT = bigp.tile([128, MC, S], f32r)     # becomes q_rot in place
            kT = bigp.tile([128, MC, S], f32r)     # becomes k_rot in place
            vT = bigp.tile([128, MC, S], f32)      # fp32: feeds PE transpose
            avT = bigp.tile([128, MC, S], f32r)    # attention output (pre o-proj)
            # [:, j, hh*65 + (0:64)] = v of head 2m+hh; col hh*65+64 = ones
            vaug = [bigp.tile([128, NJ, 130], f32r, tag=f"vaug{m}",
                              name=f"vaug{m}") for m in range(MC)]

            # ================= phase 1: q/k/v projections =================
            proj_targets = [
                ("q", qT), ("k", kT), ("v", vT),
            ]
            with (
                tc.tile_pool(name="xin", bufs=5) as xinp,
                tc.tile_pool(name="xr", bufs=5) as xrp,
                tc.tile_pool(name="psP", bufs=8, space="PSUM") as psP,
            ):
                for sq in range(4):                # S quarters of 512
                    s0 = sq * 512
                    accs = {}
                    for pname, _ in proj_targets:
                        for m in range(MC):
                            accs[(pname, m)] = psP.tile(
                                [128, 512], mybir.dt.float32, tag="acc",
                                name=f"acc_{pname}{m}")
                    for k in range(KC):
                        xs = xinp.tile([128, 512], f32)
                        dmae = nc.scalar if k % 2 == 0 else nc.gpsimd
                        dmae.dma_start(xs[:], xT_d[k * 128:(k + 1) * 128,
                                                   s0:s0 + 512])
                        xr = xrp.tile([128, 512], f32r)
                        reng = nc.gpsimd if k < 2 or k % 2 == 0 else nc.vector
                        reng.tensor_copy(xr[:], xs[:])
                        for pname, _ in proj_targets:
                            for m in range(MC):
                                nc.tensor.matmul(
                                    accs[(pname, m)][:],
                                    w_r[pname][:, k, m * 128:(m + 1) * 128],
                                    xr[:],
                                    start=(k == 0), stop=(k == KC - 1),
                                )
                    for pname, dst in proj_targets:
                        for m in range(MC):
                            nc.vector.tensor_copy(dst[:, m, s0:s0 + 512],
                                                  accs[(pname, m)][:])

            # tables arrive on the SWDGE queue behind the odd x chunks
            nc.gpsimd.dma_start(cosf[:], cosf_d[:])
            nc.gpsimd.dma_start(sinf[:], sinf_d[:])
            nc.gpsimd.dma_start(trim[:], trim_d[:])
            nc.gpsimd.dma_start(ident[:], ident_d[:])
            nc.gpsimd.dma_start(pstage[:], pmat_d[:])
            nc.vector.tensor_copy(pmat[:], pstage[:])

            # ============ phase 1.5: RoPE (in place) + v transpose ============
            with (
                tc.tile_pool(name="psSw", bufs=2, space="PSUM") as psSw,
                tc.tile_pool(name="psT", bufs=2, space="PSUM") as psT,
                tc.tile_pool(name="ropet", bufs=4) as ropet,
            ):
                for m in range(MC):
                    for src in (qT, kT):
                        for sh in range(2):
                            c0 = sh * HALF
                            sw = psSw.tile([128, HALF], mybir.dt.float32)
                            for s4 in range(2):
                                nc.tensor.matmul(
                                    sw[:, s4 * 512:(s4 + 1) * 512],
                                    pmat[:],
                                    src[:, m, c0 + s4 * 512:c0 + (s4 + 1) * 512],
                                )
                            t1 = ropet.tile([128, HALF], f32, tag="t1")
                            nc.gpsimd.tensor_mul(
                                t1[:], src[:, m, c0:c0 + HALF].bitcast(f32),
                                cosf[:, c0:c0 + HALF])
                            t2 = ropet.tile([128, HALF], f32, tag="t2")
                            nc.vector.tensor_mul(t2[:], sw[:], sinf[:, c0:c0 + HALF])
                            nc.vector.tensor_add(src[:, m, c0:c0 + HALF], t1[:], t2[:])
                # v transpose: vT [2head-dims, kv] -> v natural chunks in vaug
                for m in range(MC):
                    nc.gpsimd.tensor_copy(
                        vaug[m][:, :, 64:130:65],
                        ones_f[:, None, :].broadcast_to([128, NJ, 2]),
                    )
                for m in range(MC):
                    for j in range(NJ):
                        pt = psT.tile([128, 128], f32)
                        nc.tensor.transpose(pt[:], vT[:, m, j * 128:(j + 1) * 128],
                                            ident[:])
                        nc.scalar.copy(
                            _vaug_pair_dst(vaug, m, j),
                            pt[:].rearrange("p (hh n) -> p hh n", hh=2))

            # ---- wo load (sync queue, behind the qkv chunks) + round ----
            wost = constp.tile([128, MC * D], f32, tag="wstage")
            nc.sync.dma_start(
                wost[:].rearrange("p (c n) -> p c n", c=MC),
                woT_d.rearrange("(c p) n -> p c n", p=128),
            )
            wo_r = wres.tile([128, MC, 8, 128], f32r)
            nc.vector.tensor_copy(
                wo_r[:], wost[:].rearrange("p (c m n) -> p c m n", c=MC, m=8)
            )

            # ================= phase 2: attention =================
            with (
                tc.tile_pool(name="av", bufs=2, space="PSUM") as avp,
                tc.tile_pool(name="seg", bufs=2, space="PSUM") as segp,
                tc.tile_pool(name="attn", bufs=3) as attnp,
                tc.tile_pool(name="smax", bufs=4) as smaxp,
            ):
                def avmm(av, h, j, kv0, at, q0, jmax):
                    for sg in range(2):
                        sb0 = q0 + sg * 512
                        sb1 = sb0 + 512
                        lo = max(sb0, kv0)
                        if lo >= sb1:
                            continue
                        nc.tensor.matmul(
                            av[:, lo - q0:sb1 - q0],
                            vaug[h // 2][:, j, (h % 2) * 65:(h % 2) * 65 + 65],
                            at[:, lo - q0:sb1 - q0],
                            start=(j == 0),
                            stop=(j == min(jmax, sb1 // 128) - 1),
                        )

                for m in range(MC):
                    for hh in range(2):
                        h = 2 * m + hh
                        p0 = hh * DK
                        for H in range(2):
                            q0 = H * HALF
                            jmax = (H + 1) * (NJ // 2)
                            av = avp.tile([DK + 1, HALF], mybir.dt.float32)
                            pend = []
                            for j in range(jmax):
                                kv0 = j * 128
                                at = attnp.tile([128, HALF], f32r)
                                jlo = max(q0, kv0)            # valid q start
                                sc = segp.tile([128, HALF], mybir.dt.float32)
                                for sg in range(2):
                                    sb0 = q0 + sg * 512       # seg bounds (abs cols)
                                    sb1 = sb0 + 512
                                    lo = max(sb0, kv0)
                                    if lo >= sb1:
                                        continue
                                    nc.tensor.matmul(
                                        sc[:, lo - q0:sb1 - q0],
                                        kT[p0:p0 + DK, m, kv0:kv0 + 128],
                                        qT[p0:p0 + DK, m, lo:lo + sb1 - lo],
                                    )
                                nc.scalar.activation(
                                    at[:, jlo - q0:HALF],
                                    sc[:, jlo - q0:HALF], EXP, scale=0.125)
                                if jlo == kv0:                # diagonal block
                                    dg = at[:, jlo - q0:jlo - q0 + 128]
                                    meng = nc.gpsimd if hh == 0 else nc.vector
                                    meng.tensor_mul(
                                        dg, dg.bitcast(f32), trim[:])
                                pend.append((h, j, kv0, at))
                                while pend:
                                    avmm(av, *pend.pop(0), q0, jmax)
                            rec = smaxp.tile([1, HALF], f32, tag="rec")
                            nc.vector.reciprocal(rec[:], av[DK:DK + 1, :])
                            bc = smaxp.tile([DK, HALF], f32, tag="bc")
                            nc.gpsimd.partition_broadcast(bc[:], rec[:])
                            nc.vector.tensor_mul(
                                avT[p0:p0 + DK, m, q0:q0 + HALF],
                                av[0:DK, :], bc[:])

            # ================= phase 3: output projection =================
            with (
                tc.tile_pool(name="psO", bufs=2, space="PSUM") as psO,
                tc.tile_pool(name="ost", bufs=3) as ostp,
            ):
                for mo in range(8):
                    for sh in range(2):
                        c0 = sh * HALF
                        po = psO.tile([128, HALF], mybir.dt.float32)
                        for c in range(MC):
                            for s4 in range(2):
                                nc.tensor.matmul(
                                    po[:, s4 * 512:(s4 + 1) * 512],
                                    wo_r[:, c, mo, :],
                                    avT[:, c, c0 + s4 * 512:c0 + (s4 + 1) * 512],
                                    start=(c == 0), stop=(c == MC - 1),
                                )
                        ot = ostp.tile([128, HALF], f32)
                        if (2 * mo + sh) % 2 == 0:
                            nc.scalar.copy(ot[:], po[:])
                        else:
                            nc.vector.tensor_copy(ot[:], po[:])
                        nc.sync.dma_start(
                            outp_d[mo * 128:(mo + 1) * 128, c0:c0 + HALF], ot[:])

    nc.compile()
    return nc


def get_program():
    if "nc" not in _CACHE:
        _CACHE["nc"] = _build_program()
    return _CACHE["nc"]


def make_in_maps(x, wq, wk, wv, wo, token_positions):
    cosf, sinf, pmat, ident, trimask = _host_tables(token_positions)
    x = np.asarray(x, dtype=np.float32)
    wq = np.asarray(wq, dtype=np.float32)
    wk = np.asarray(wk, dtype=np.float32)
    wv = np.asarray(wv, dtype=np.float32)
    wo = np.asarray(wo, dtype=np.float32)
    in_maps = []
    for g in range(NCORES):
        b = g // GROUP
        hg = g % GROUP
        sl = slice(hg * DQ, (hg + 1) * DQ)
        in_maps.append({
            "xT": np.ascontiguousarray(x[b].T),
            "wqT": np.ascontiguousarray(wq[sl, :].T),
            "wkT": np.ascontiguousarray(wk[sl, :].T),
            "wvT": np.ascontiguousarray(wv[sl, :].T),
            "woT": np.ascontiguousarray(wo[:, sl].T),
            "cosf": cosf, "sinf": sinf, "pmat": pmat,
            "ident": ident, "trimask": trimask,
        })
    return in_maps


def kernel(x, token_positions, wq, wk, wv, wo):
    from concourse.bass_utils import run_bass_kernel_spmd

    # token_positions is always arange(S) for this problem size; the rope
    # tables are built from it on the host either way.
    tp = np.asarray(token_positions)
    assert tp.shape == (S,)

    nc = get_program()
    in_maps = make_in_maps(x, wq, wk, wv, wo, tp)
    res = run_bass_kernel_spmd(nc, in_maps, list(range(NCORES)))
    out = np.zeros((B, S, D), dtype=np.float32)
    for b in range(B):
        acc = np.zeros((D, S), dtype=np.float32)
        for g in range(b * GROUP, (b + 1) * GROUP):
            acc += res.results[g]["outp"]
        out[b] = acc.T
    return out


# revision 38
# speedup vs baseline: 1.0931x; 1.0516x over previous
"""Causal multi-head self-attention with RoPE on 8 Trainium2 NeuronCores.

Sharding: data-parallel over batch (B=2 -> 2 groups of 4 cores), tensor-
parallel over heads within each group (16 heads -> 4 heads/core). Each core
computes q/k/v projections for its 4 heads, RoPE, causal attention, and a
partial output projection; partials are summed across the 4 cores of a
batch group (host-side reduction in v1).

Math layout notes (per core):
  - everything is kept "transposed": xT [D, S], qT/kT [256, S], so matmuls
    contract over the partition dim with weights stationary.
  - scores are computed transposed, scT[kv, q], so softmax-exp feeds the
    AV matmul directly (no attention-matrix transpose). The softmax
    denominator comes from a ones-column appended to V. Max-subtraction is
    skipped: logits are bounded (|logit| < ~20) so exp is safe in fp32.
  - matmuls run in float32r (~1.6e-4 relerr, 4x faster than fp32 on PE).
"""

import numpy as np

B = 2
S = 2048
D = 1024
NH = 16
DK = 64
THETA = 10000.0
NCORES = 8
GROUP = 4          # cores per batch group (tensor-parallel over heads)
DQ = 256           # head dims per core (4 heads x 64)
NEG = -1.0e9

_CACHE = {}


def _host_tables(pos):
    pos = np.asarray(pos, dtype=np.float64)
    half = np.arange(0, DK, 2, dtype=np.float64) / DK          # (32,)
    inv_freq = 1.0 / (THETA ** half)                           # (32,)
    ang = pos[:, None] * inv_freq[None, :]                     # (S, 32)
    cos = np.cos(ang)
    sin = np.sin(ang)
    d = np.arange(128)
    ip = (d % DK) // 2                                         # pair index per row
    cosf = cos[:, ip].T.astype(np.float32).copy()              # (128, S)
    sinf = sin[:, ip].T.astype(np.float32).copy()
    pmat = np.zeros((128, 128), dtype=np.float32)
    for i in range(64):
        pmat[2 * i + 1, 2 * i] = -1.0                          # qswap[2i]   = -q[2i+1]
        pmat[2 * i, 2 * i + 1] = 1.0                           # qswap[2i+1] = +q[2i]
    ident = np.eye(128, dtype=np.float32)
    r = np.arange(128)
    trimask = (r[None, :] >= r[:, None]).astype(np.float32)
    return cosf, sinf, pmat, ident, trimask


def _vaug_pair_dst(vaug, m, j):
    return vaug[m][:, j, 0:130].rearrange(
        "p (hh n) -> p hh n", hh=2)[:, :, 0:64]

def _build_program():
    import concourse.bacc as bacc
    import concourse.mybir as mybir
    import concourse.tile as tile

    f32 = mybir.dt.float32
    f32r = mybir.dt.float32r
    EXP = mybir.ActivationFunctionType.Exp

    nc = bacc.Bacc("TRN2", target_bir_lowering=False)

    xT_d = nc.dram_tensor("xT", [D, S], f32, kind="ExternalInput")
    wqT_d = nc.dram_tensor("wqT", [D, DQ], f32, kind="ExternalInput")
    wkT_d = nc.dram_tensor("wkT", [D, DQ], f32, kind="ExternalInput")
    wvT_d = nc.dram_tensor("wvT", [D, DQ], f32, kind="ExternalInput")
    woT_d = nc.dram_tensor("woT", [DQ, D], f32, kind="ExternalInput")
    cosf_d = nc.dram_tensor("cosf", [128, S], f32, kind="ExternalInput")
    sinf_d = nc.dram_tensor("sinf", [128, S], f32, kind="ExternalInput")
    pmat_d = nc.dram_tensor("pmat", [128, 128], f32, kind="ExternalInput")
    ident_d = nc.dram_tensor("ident", [128, 128], f32, kind="ExternalInput")
    trim_d = nc.dram_tensor("trimask", [128, 128], f32, kind="ExternalInput")
    outp_d = nc.dram_tensor("outp", [D, S], f32, kind="ExternalOutput")

    KC = D // 128       # 8 contraction chunks
    MC = DQ // 128      # 2 head-dim chunks per core
    NJ = S // 128       # 16 kv chunks
    HALF = S // 2       # 1024

    with tile.TileContext(nc) as tc:
        with (
            tc.tile_pool(name="const", bufs=1) as constp,
            tc.tile_pool(name="wres", bufs=1) as wres,
            tc.tile_pool(name="big", bufs=1) as bigp,
        ):
            # ---- constants ----
            cosf = constp.tile([128, S], f32)
            sinf = constp.tile([128, S], f32)
            trim = constp.tile([128, 128], f32)
            ident = constp.tile([128, 128], f32)
            pstage = constp.tile([128, 128], f32)
            pmat = constp.tile([128, 128], f32r)
            ones_f = constp.tile([128, 1], f32)
            nc.vector.memset(ones_f[:], 1.0)

            # ---- weights: chunked load + round so k=0 matmuls start early
            w_r = {}
            wst = {}
            for name, dram in (("q", wqT_d), ("k", wkT_d), ("v", wvT_d)):
                wst[name] = constp.tile([128, KC * DQ], f32, tag=f"wst{name}",
                                        name=f"wst_{name}")
                w_r[name] = wres.tile([128, KC, DQ], f32r, tag=f"w{name}",
                                      name=f"w_{name}")
            for k in range(KC):
                for name, dram in (("q", wqT_d), ("k", wkT_d), ("v", wvT_d)):
                    st = wst[name][:].rearrange("p (k n) -> p k n", k=KC)
                    nc.sync.dma_start(st[:, k, :],
                                      dram[k * 128:(k + 1) * 128, :])
                    nc.vector.tensor_copy(w_r[name][:, k, :], st[:, k, :])

            # ---- resident activations ----
            qT = bigp.tile([128, MC, S], f32r)     # becomes q_rot in place
            kT = bigp.tile([128, MC, S], f32r)     # becomes k_rot in place
            vT = bigp.tile([128, MC, S], f32)      # fp32: feeds PE transpose
            avT = bigp.tile([128, MC, S], f32r)    # attention output (pre o-proj)
            # [:, j, hh*65 + (0:64)] = v of head 2m+hh; col hh*65+64 = ones
            vaug = [bigp.tile([128, NJ, 130], f32r, tag=f"vaug{m}",
                              name=f"vaug{m}") for m in range(MC)]

            # ================= phase 1: q/k/v projections =================
            proj_targets = [
                ("q", qT), ("k", kT), ("v", vT),
            ]
            with (
                tc.tile_pool(name="xin", bufs=5) as xinp,
                tc.tile_pool(name="xr", bufs=5) as xrp,
                tc.tile_pool(name="psP", bufs=8, space="PSUM") as psP,
            ):
                for sq in range(4):                # S quarters of 512
                    s0 = sq * 512
                    accs = {}
                    for pname, _ in proj_targets:
                        for m in range(MC):
                            accs[(pname, m)] = psP.tile(
                                [128, 512], mybir.dt.float32, tag="acc",
                                name=f"acc_{pname}{m}")
                    for k in range(KC):
                        xs = xinp.tile([128, 512], f32)
                        dmae = nc.scalar if k % 2 == 0 else nc.gpsimd
                        dmae.dma_start(xs[:], xT_d[k * 128:(k + 1) * 128,
                                                   s0:s0 + 512])
                        xr = xrp.tile([128, 512], f32r)
                        reng = nc.gpsimd if k < 2 or k % 2 == 0 else nc.vector
                        reng.tensor_copy(xr[:], xs[:])
                        for pname, _ in proj_targets:
                            for m in range(MC):
                                nc.tensor.matmul(
                                    accs[(pname, m)][:],
                                    w_r[pname][:, k, m * 128:(m + 1) * 128],
                                    xr[:],
                                    start=(k == 0), stop=(k == KC - 1),
                                )
                    for pname, dst in proj_targets:
                        for m in range(MC):
                            nc.vector.tensor_copy(dst[:, m, s0:s0 + 512],
                                                  accs[(pname, m)][:])

            # tables arrive on the SWDGE queue behind the odd x chunks
            nc.gpsimd.dma_start(cosf[:], cosf_d[:])
            nc.gpsimd.dma_start(sinf[:], sinf_d[:])
            nc.gpsimd.dma_start(trim[:], trim_d[:])
            nc.gpsimd.dma_start(ident[:], ident_d[:])
            nc.gpsimd.dma_start(pstage[:], pmat_d[:])
            nc.vector.tensor_copy(pmat[:], pstage[:])

            # ============ phase 1.5: RoPE (in place) + v transpose ============
            with (
                tc.tile_pool(name="psSw", bufs=2, space="PSUM") as psSw,
                tc.tile_pool(name="psT", bufs=2, space="PSUM") as psT,
                tc.tile_pool(name="ropet", bufs=4) as ropet,
            ):
                for m in range(MC):
                    for src in (qT, kT):
                        for sh in range(2):
                            c0 = sh * HALF
                            sw = psSw.tile([128, HALF], mybir.dt.float32)
                            for s4 in range(2):
                                nc.tensor.matmul(
                                    sw[:, s4 * 512:(s4 + 1) * 512],
                                    pmat[:],
                                    src[:, m, c0 + s4 * 512:c0 + (s4 + 1) * 512],
                                )
                            t1 = ropet.tile([128, HALF], f32, tag="t1")
                            nc.gpsimd.tensor_mul(
                                t1[:], src[:, m, c0:c0 + HALF].bitcast(f32),
                                cosf[:, c0:c0 + HALF])
                            t2 = ropet.tile([128, HALF], f32, tag="t2")
                            nc.vector.tensor_mul(t2[:], sw[:], sinf[:, c0:c0 + HALF])
                            nc.vector.tensor_add(src[:, m, c0:c0 + HALF], t1[:], t2[:])
                # v transpose: vT [2head-dims, kv] -> v natural chunks in vaug
                for m in range(MC):
                    nc.gpsimd.tensor_copy(
                        vaug[m][:, :, 64:130:65],
                        ones_f[:, None, :].broadcast_to([128, NJ, 2]),
                    )
                for m in range(MC):
                    for j in range(NJ):
                        pt = psT.tile([128, 128], f32)
                        nc.tensor.transpose(pt[:], vT[:, m, j * 128:(j + 1) * 128],
                                            ident[:])
                        nc.scalar.copy(
                            _vaug_pair_dst(vaug, m, j),
                            pt[:].rearrange("p (hh n) -> p hh n", hh=2))

            # ---- wo load (sync queue, behind the qkv chunks) + round ----
            wost = constp.tile([128, MC * D], f32, tag="wstage")
            nc.sync.dma_start(
                wost[:].rearrange("p (c n) -> p c n", c=MC),
                woT_d.rearrange("(c p) n -> p c n", p=128),
            )
            wo_r = wres.tile([128, MC, 8, 128], f32r)
            nc.vector.tensor_copy(
                wo_r[:], wost[:].rearrange("p (c m n) -> p c m n", c=MC, m=8)
            )

            # ================= phase 2: attention =================
            with (
                tc.tile_pool(name="av", bufs=2, space="PSUM") as avp,
                tc.tile_pool(name="seg", bufs=2, space="PSUM") as segp,
                tc.tile_pool(name="attn", bufs=3) as attnp,
                tc.tile_pool(name="smax", bufs=2) as smaxp,
                tc.tile_pool(name="ost", bufs=3) as ostp,
            ):
                oq = [(mo, sh) for sh in range(2) for mo in range(8)]

                def emit_oproj_unit():
                    mo, sh = oq.pop(0)
                    c0 = sh * HALF
                    # po reuses a freed av slot: same 2-bank footprint
                    po = avp.tile([128, HALF], mybir.dt.float32, tag="av",
                                  name=f"po{mo}_{sh}")
                    for c in range(MC):
                        for s4 in range(2):
                            nc.tensor.matmul(
                                po[:, s4 * 512:(s4 + 1) * 512],
                                wo_r[:, c, mo, :],
                                avT[:, c, c0 + s4 * 512:c0 + (s4 + 1) * 512],
                                start=(c == 0), stop=(c == MC - 1),
                            )
                    ot = ostp.tile([128, HALF], f32, tag="ot",
                                   name=f"ot{mo}_{sh}")
                    if (2 * mo + sh) % 2 == 0:
                        nc.scalar.copy(ot[:], po[:])
                    else:
                        nc.vector.tensor_copy(ot[:], po[:])
                    nc.sync.dma_start(
                        outp_d[mo * 128:(mo + 1) * 128, c0:c0 + HALF], ot[:])

                def avmm(av, h, j, kv0, at, q0, jmax):
                    for sg in range(2):
                        sb0 = q0 + sg * 512
                        sb1 = sb0 + 512
                        lo = max(sb0, kv0)
                        if lo >= sb1:
                            continue
                        nc.tensor.matmul(
                            av[:, lo - q0:sb1 - q0],
                            vaug[h // 2][:, j, (h % 2) * 65:(h % 2) * 65 + 65],
                            at[:, lo - q0:sb1 - q0],
                            start=(j == 0),
                            stop=(j == min(jmax, sb1 // 128) - 1),
                        )

                for m in range(MC):
                    for hh in range(2):
                        h = 2 * m + hh
                        p0 = hh * DK
                        for H in range(2):
                            q0 = H * HALF
                            jmax = (H + 1) * (NJ // 2)
                            av = avp.tile([DK + 1, HALF], mybir.dt.float32)
                            pend = []
                            for j in range(jmax):
                                kv0 = j * 128
                                at = attnp.tile([128, HALF], f32r)
                                jlo = max(q0, kv0)            # valid q start
                                sc = segp.tile([128, HALF], mybir.dt.float32)
                                for sg in range(2):
                                    sb0 = q0 + sg * 512       # seg bounds (abs cols)
                                    sb1 = sb0 + 512
                                    lo = max(sb0, kv0)
                                    if lo >= sb1:
                                        continue
                                    nc.tensor.matmul(
                                        sc[:, lo - q0:sb1 - q0],
                                        kT[p0:p0 + DK, m, kv0:kv0 + 128],
                                        qT[p0:p0 + DK, m, lo:lo + sb1 - lo],
                                    )
                                nc.scalar.activation(
                                    at[:, jlo - q0:HALF],
                                    sc[:, jlo - q0:HALF], EXP, scale=0.125)
                                if jlo == kv0:                # diagonal block
                                    dg = at[:, jlo - q0:jlo - q0 + 128]
                                    meng = nc.gpsimd if hh == 0 else nc.vector
                                    meng.tensor_mul(
                                        dg, dg.bitcast(f32), trim[:])
                                pend.append((h, j, kv0, at))
                                while pend:
                                    avmm(av, *pend.pop(0), q0, jmax)
                                if (m == MC - 1 and hh == 1 and H == 1
                                        and j % 2 == 1 and j >= 3 and oq
                                        and oq[0][1] == 0):
                                    emit_oproj_unit()
                            rec = smaxp.tile([1, HALF], f32, tag="rec")
                            nc.vector.reciprocal(rec[:], av[DK:DK + 1, :])
                            bc = smaxp.tile([DK, HALF], f32, tag="bc")
                            nc.gpsimd.partition_broadcast(bc[:], rec[:])
                            nc.vector.tensor_mul(
                                avT[p0:p0 + DK, m, q0:q0 + HALF],
                                av[0:DK, :], bc[:])

                # ======== phase 3: remaining output-projection units ========
                while oq:
                    emit_oproj_unit()

    nc.compile()
    return nc


def get_program():
    if "nc" not in _CACHE:
        _CACHE["nc"] = _build_program()
    return _CACHE["nc"]


def make_in_maps(x, wq, wk, wv, wo, token_positions):
    cosf, sinf, pmat, ident, trimask = _host_tables(token_positions)
    x = np.asarray(x, dtype=np.float32)
    wq = np.asarray(wq, dtype=np.float32)
    wk = np.asarray(wk, dtype=np.float32)
    wv = np.asarray(wv, dtype=np.float32)
    wo = np.asarray(wo, dtype=np.float32)
    in_maps = []
    for g in range(NCORES):
        b = g // GROUP
        hg = g % GROUP
        sl = slice(hg * DQ, (hg + 1) * DQ)
        in_maps.append({
            "xT": np.ascontiguousarray(x[b].T),
            "wqT": np.ascontiguousarray(wq[sl, :].T),
            "wkT": np.ascontiguousarray(wk[sl, :].T),
            "wvT": np.ascontiguousarray(wv[sl, :].T),
            "woT": np.ascontiguousarray(wo[:, sl].T),
            "cosf": cosf, "sinf": sinf, "pmat": pmat,
            "ident": ident, "trimask": trimask,
        })
    return in_maps


def kernel(x, token_positions, wq, wk, wv, wo):
    from concourse.bass_utils import run_bass_kernel_spmd

    # token_positions is always arange(S) for this problem size; the rope
    # tables are built from it on the host either way.
    tp = np.asarray(token_positions)
    assert tp.shape == (S,)

    nc = get_program()
    in_maps = make_in_maps(x, wq, wk, wv, wo, tp)
    res = run_bass_kernel_spmd(nc, in_maps, list(range(NCORES)))
    out = np.zeros((B, S, D), dtype=np.float32)
    for b in range(B):
        acc = np.zeros((D, S), dtype=np.float32)
        for g in range(b * GROUP, (b + 1) * GROUP):
            acc += res.results[g]["outp"]
        out[b] = acc.T
    return out
